# revision 7
# baseline (speedup 1.0000x reference)
"""Trainium2 Bass kernel for nn_Attention_85813446574600.

Reference computes:
    s_x = x @ W[:F] + b            # [B,T,1]
    s_c = context @ W[F:]          # [C,1]
    scores = s_x + s_c             # [B,T,C,1]
    att = softmax(scores, axis=-1) # softmax over a SIZE-1 axis -> exactly 1.0
    out = einsum('btc,btf->bcf', att, x)

Since softmax over the last (size-1) axis is identically 1.0 for any finite
scores, the output is exactly out[b,c,f] = sum_t x[b,t,f], independent of c
(and of context/W/b entirely).

Device kernel (per core, batch-sharded 32/8 = 4 batches), raw Bass (no Tile
framework -- avoids its entry/exit barrier overhead):

  sync+scalar   : input DMAs split across BOTH HWDGE rings (qSP: b0,b1;
                  qAct: b2,b3) so the combined stream approaches the per-core
                  HBM limit. Partition p holds consecutive T rows, giving
                  per-partition-contiguous 8KB descriptors. The last batch is
                  split into two half-loads so its reduction starts while the
                  second half is still streaming. Output DMAs are interleaved
                  on both rings behind the inputs: one 512KB DMA per batch
                  with 4KB-contiguous descriptors.
  vector engine : pre-reduce the T rows in each partition with wide adds,
                  then copy the matmul result PSUM->SBUF duplicated twice
                  side-by-side (stride-0 read) to feed the 4KB-descriptor
                  output DMA.
  tensor engine : ONES[128,128] @ total -> PSUM; an all-ones stationary
                  matrix both sums across partitions and broadcasts the
                  result to all 128 output partitions in one matmul.
"""

import sys

for _p in ("/opt/trn_rl_repo",):
    if _p not in sys.path:
        sys.path.insert(0, _p)

import dataclasses
from contextlib import ExitStack

import numpy as np

import concourse.bass as bass
import concourse.mybir as mybir
from concourse.bass_utils import run_bass_kernel_spmd

# Problem shapes (hardcoded per harness contract)
B, T, C, F = 32, 512, 256, 512
N_CORES = 8
B_LOC = B // N_CORES  # 4 batches per core
P = 128               # SBUF/PSUM partitions
TT = T // P           # 4 T-rows folded into each partition
DT = mybir.dt.float32

_NC_CACHE = {}


def _dup2(ap):
    """View `ap` ([128, N]) as [128, 2, N] reading the same data twice
    (stride-0 middle dim)."""
    a = ap.ap
    return dataclasses.replace(
        ap, ap=type(a)([list(a[0]), [0, 2], list(a[1])])
    )


def _build_nc():
    nc = bass.Bass("TRN2", target_bir_lowering=False)
    x = nc.dram_tensor("x", [B_LOC, T, F], DT, kind="ExternalInput").ap()
    out = nc.dram_tensor("out", [B_LOC, C, F], DT, kind="ExternalOutput").ap()

    with ExitStack() as ctx:
        ec = ctx.enter_context
        ones = ec(nc.sbuf_tensor("ones", [P, P], DT)).ap()
        # b0..b2: one [128, 4*F] tile each; b3: two [128, 2*F] half tiles
        xts = [
            ec(nc.sbuf_tensor(f"xt{b}", [P, TT * F], DT)).ap() for b in range(3)
        ]
        xt3a = ec(nc.sbuf_tensor("xt3a", [P, 2 * F], DT)).ap()
        xt3b = ec(nc.sbuf_tensor("xt3b", [P, 2 * F], DT)).ap()
        pairs = [
            ec(nc.sbuf_tensor(f"pair{b}", [P, 2 * F], DT)).ap() for b in range(3)
        ]
        t3a = ec(nc.sbuf_tensor("t3a", [P, F], DT)).ap()
        t3b = ec(nc.sbuf_tensor("t3b", [P, F], DT)).ap()
        totals = [
            ec(nc.sbuf_tensor(f"total{b}", [P, F], DT)).ap() for b in range(B_LOC)
        ]
        # duplicated result slabs: [128, 2*F] so the out DMA sees 4KB
        # contiguous per partition
        ots = [
            ec(nc.sbuf_tensor(f"ot{b}", [P, 2 * F], DT)).ap() for b in range(B_LOC)
        ]
        accs = [ec(nc.psum_tensor(f"acc{b}", [P, F], DT)).ap() for b in range(B_LOC)]

        in_sems = [ec(nc.semaphore(f"in_sem{b}")) for b in range(3)]
        in3a_sem = ec(nc.semaphore("in3a_sem"))
        in3b_sem = ec(nc.semaphore("in3b_sem"))
        vec_sem = ec(nc.semaphore("vec_sem"))
        vv_sem = ec(nc.semaphore("vv_sem"))
        pe_sem = ec(nc.semaphore("pe_sem"))
        cp_sem = ec(nc.semaphore("cp_sem"))
        osem_sp = ec(nc.semaphore("osem_sp"))
        osem_act = ec(nc.semaphore("osem_act"))

        block = ec(nc.Block())

        def in_dma(eng, b):
            # partition p <- x[b, TT*p : TT*(p+1), :], contiguous 8KB/partition
            src = x[b].rearrange("(p l) f -> p l f", p=P)
            return eng.dma_start(
                xts[b].rearrange("p (l f) -> p l f", l=TT), src
            ).then_inc(in_sems[b], 16)

        def out_dma(eng, b, sem):
            # out[b] rows (2p, 2p+1) <- ot[b] partition p (4KB contiguous)
            dst = out[b].rearrange("(p r) f -> p r f", p=P)
            src = ots[b].rearrange("p (r f) -> p r f", r=2)
            return eng.dma_start(dst, src).then_inc(sem, 16)

        @block.sync
        def _(sync):
            in_dma(sync, 0)
            in_dma(sync, 1)
            sync.wait_ge(cp_sem, 1)
            out_dma(sync, 0, osem_sp)
            sync.wait_ge(cp_sem, 3)
            out_dma(sync, 2, osem_sp)
            sync.wait_ge(osem_sp, 32)

        @block.scalar
        def _(scalar):
            in_dma(scalar, 2)
            src3 = x[3].rearrange("(h p l) f -> h p l f", h=2, p=P)
            scalar.dma_start(
                xt3a.rearrange("p (l f) -> p l f", l=2), src3[0]
            ).then_inc(in3a_sem, 16)
            scalar.dma_start(
                xt3b.rearrange("p (l f) -> p l f", l=2), src3[1]
            ).then_inc(in3b_sem, 16)
            scalar.wait_ge(cp_sem, 2)
            out_dma(scalar, 1, osem_act)
            scalar.wait_ge(cp_sem, 4)
            out_dma(scalar, 3, osem_act)
            scalar.wait_ge(osem_act, 32)

        @block.vector
        def _(vector):
            nc.vector.memset(ones, 1.0).then_inc(vec_sem, 1)

            def adds(b):
                vector.wait_ge(in_sems[b], 16)
                nc.vector.tensor_add(
                    pairs[b], xts[b][:, 0 : 2 * F], xts[b][:, 2 * F : 4 * F]
                ).then_inc(vv_sem, 1)
                # same-engine RAW: the DVE pipeline is deep, so the dependent
                # read must wait on the writer's semaphore
                vector.wait_ge(vv_sem, b + 1)
                nc.vector.tensor_add(
                    totals[b], pairs[b][:, 0:F], pairs[b][:, F : 2 * F]
                ).then_inc(vec_sem, 1)

            def copy(b):
                vector.wait_ge(pe_sem, b + 1)
                nc.vector.tensor_copy(ots[b], _dup2(accs[b])).then_inc(cp_sem, 1)

            adds(0)
            adds(1)
            copy(0)
            adds(2)
            copy(1)
            # batch 3: two half reductions pipelined with its streaming load
            vector.wait_ge(in3a_sem, 16)
            nc.vector.tensor_add(t3a, xt3a[:, 0:F], xt3a[:, F : 2 * F]).then_inc(
                vv_sem, 1
            )
            vector.wait_ge(in3b_sem, 16)
            nc.vector.tensor_add(t3b, xt3b[:, 0:F], xt3b[:, F : 2 * F]).then_inc(
                vv_sem, 1
            )
            vector.wait_ge(vv_sem, 5)
            nc.vector.tensor_add(totals[3], t3a, t3b).then_inc(vec_sem, 1)
            copy(2)
            copy(3)

        @block.tensor
        def _(tensor):
            for b in range(B_LOC):
                tensor.wait_ge(vec_sem, b + 2)
                nc.tensor.matmul(
                    accs[b], ones, totals[b], start=True, stop=True
                ).then_inc(pe_sem, 1)

    return nc


def _get_nc():
    if "nc" not in _NC_CACHE:
        _NC_CACHE["nc"] = _build_nc()
    return _NC_CACHE["nc"]


def kernel(x, context=None, W=None, b=None, **_unused):
    """Full inputs in, full output out. context/W/b provably do not affect
    the output (softmax over a size-1 axis is identically 1)."""
    x = np.ascontiguousarray(np.asarray(x), dtype=np.float32)
    assert x.shape == (B, T, F), x.shape

    nc = _get_nc()
    in_maps = [{"x": x[i * B_LOC : (i + 1) * B_LOC]} for i in range(N_CORES)]
    res = run_bass_kernel_spmd(nc, in_maps, core_ids=list(range(N_CORES)))
    return np.concatenate([r["out"] for r in res.results], axis=0)


# revision 8
# speedup vs baseline: 1.0842x; 1.0842x over previous
"""Trainium2 Bass kernel for nn_Attention_85813446574600.

Reference computes:
    s_x = x @ W[:F] + b            # [B,T,1]
    s_c = context @ W[F:]          # [C,1]
    scores = s_x + s_c             # [B,T,C,1]
    att = softmax(scores, axis=-1) # softmax over a SIZE-1 axis -> exactly 1.0
    out = einsum('btc,btf->bcf', att, x)

Since softmax over the last (size-1) axis is identically 1.0 for any finite
scores, the output is exactly out[b,c,f] = sum_t x[b,t,f], independent of c
(and of context/W/b entirely).

Device kernel (per core, batch-sharded 32/8 = 4 batches), raw Bass (no Tile
framework -- avoids its entry/exit barrier overhead):

  sync engine   : all input DMAs on the qSP HWDGE ring (two concurrent
                  rings were measured SLOWER: 2x146GB/s vs 1x323GB/s).
                  Partition p holds consecutive T rows, giving
                  per-partition-contiguous 8KB descriptors. The last batch is
                  split into two half-loads so its reduction starts while the
                  second half is still streaming.
  scalar engine : one 512KB output DMA per batch on the qAct ring with
                  4KB-contiguous descriptors.
  vector engine : pre-reduce the T rows in each partition with wide adds,
                  then copy the matmul result PSUM->SBUF duplicated twice
                  side-by-side (stride-0 read) to feed the 4KB-descriptor
                  output DMA.
  tensor engine : ONES[128,128] @ total -> PSUM; an all-ones stationary
                  matrix both sums across partitions and broadcasts the
                  result to all 128 output partitions in one matmul.
"""

import sys

for _p in ("/opt/trn_rl_repo",):
    if _p not in sys.path:
        sys.path.insert(0, _p)

import dataclasses
from contextlib import ExitStack

import numpy as np

import concourse.bass as bass
import concourse.mybir as mybir
from concourse.bass_utils import run_bass_kernel_spmd

# Problem shapes (hardcoded per harness contract)
B, T, C, F = 32, 512, 256, 512
N_CORES = 8
B_LOC = B // N_CORES  # 4 batches per core
P = 128               # SBUF/PSUM partitions
TT = T // P           # 4 T-rows folded into each partition
DT = mybir.dt.float32

_NC_CACHE = {}


def _dup2(ap):
    """View `ap` ([128, N]) as [128, 2, N] reading the same data twice
    (stride-0 middle dim)."""
    a = ap.ap
    return dataclasses.replace(
        ap, ap=type(a)([list(a[0]), [0, 2], list(a[1])])
    )


def _build_nc():
    nc = bass.Bass("TRN2", target_bir_lowering=False)
    x = nc.dram_tensor("x", [B_LOC, T, F], DT, kind="ExternalInput").ap()
    out = nc.dram_tensor("out", [B_LOC, C, F], DT, kind="ExternalOutput").ap()

    with ExitStack() as ctx:
        ec = ctx.enter_context
        ones = ec(nc.sbuf_tensor("ones", [P, P], DT)).ap()
        # b0..b2: one [128, 4*F] tile each; b3: two [128, 2*F] half tiles
        xts = [
            ec(nc.sbuf_tensor(f"xt{b}", [P, TT * F], DT)).ap() for b in range(3)
        ]
        xt3a = ec(nc.sbuf_tensor("xt3a", [P, 2 * F], DT)).ap()
        xt3b = ec(nc.sbuf_tensor("xt3b", [P, 2 * F], DT)).ap()
        pairs = [
            ec(nc.sbuf_tensor(f"pair{b}", [P, 2 * F], DT)).ap() for b in range(3)
        ]
        t3a = ec(nc.sbuf_tensor("t3a", [P, F], DT)).ap()
        t3b = ec(nc.sbuf_tensor("t3b", [P, F], DT)).ap()
        totals = [
            ec(nc.sbuf_tensor(f"total{b}", [P, F], DT)).ap() for b in range(B_LOC)
        ]
        # duplicated result slabs: [128, 2*F] so the out DMA sees 4KB
        # contiguous per partition
        ots = [
            ec(nc.sbuf_tensor(f"ot{b}", [P, 2 * F], DT)).ap() for b in range(B_LOC)
        ]
        accs = [ec(nc.psum_tensor(f"acc{b}", [P, F], DT)).ap() for b in range(B_LOC)]

        in_sems = [ec(nc.semaphore(f"in_sem{b}")) for b in range(3)]
        in3a_sem = ec(nc.semaphore("in3a_sem"))
        in3b_sem = ec(nc.semaphore("in3b_sem"))
        vec_sem = ec(nc.semaphore("vec_sem"))
        vv_sem = ec(nc.semaphore("vv_sem"))
        pe_sem = ec(nc.semaphore("pe_sem"))
        cp_sem = ec(nc.semaphore("cp_sem"))
        osem_sp = ec(nc.semaphore("osem_sp"))
        osem_act = ec(nc.semaphore("osem_act"))

        block = ec(nc.Block())

        def in_dma(eng, b):
            # partition p <- x[b, TT*p : TT*(p+1), :], contiguous 8KB/partition
            src = x[b].rearrange("(p l) f -> p l f", p=P)
            return eng.dma_start(
                xts[b].rearrange("p (l f) -> p l f", l=TT), src
            ).then_inc(in_sems[b], 16)

        def out_dma(eng, b, sem):
            # out[b] rows (2p, 2p+1) <- ot[b] partition p (4KB contiguous)
            dst = out[b].rearrange("(p r) f -> p r f", p=P)
            src = ots[b].rearrange("p (r f) -> p r f", r=2)
            return eng.dma_start(dst, src).then_inc(sem, 16)

        @block.sync
        def _(sync):
            in_dma(sync, 0)
            in_dma(sync, 1)
            in_dma(sync, 2)
            src3 = x[3].rearrange("(h p l) f -> h p l f", h=2, p=P)
            sync.dma_start(
                xt3a.rearrange("p (l f) -> p l f", l=2), src3[0]
            ).then_inc(in3a_sem, 16)
            sync.dma_start(
                xt3b.rearrange("p (l f) -> p l f", l=2), src3[1]
            ).then_inc(in3b_sem, 16)

        @block.scalar
        def _(scalar):
            for b in range(B_LOC):
                scalar.wait_ge(cp_sem, b + 1)
                out_dma(scalar, b, osem_act)
            scalar.wait_ge(osem_act, 16 * B_LOC)

        @block.vector
        def _(vector):
            nc.vector.memset(ones, 1.0).then_inc(vec_sem, 1)

            def adds(b):
                vector.wait_ge(in_sems[b], 16)
                nc.vector.tensor_add(
                    pairs[b], xts[b][:, 0 : 2 * F], xts[b][:, 2 * F : 4 * F]
                ).then_inc(vv_sem, 1)
                # same-engine RAW: the DVE pipeline is deep, so the dependent
                # read must wait on the writer's semaphore
                vector.wait_ge(vv_sem, b + 1)
                nc.vector.tensor_add(
                    totals[b], pairs[b][:, 0:F], pairs[b][:, F : 2 * F]
                ).then_inc(vec_sem, 1)

            def copy(b):
                vector.wait_ge(pe_sem, b + 1)
                nc.vector.tensor_copy(ots[b], _dup2(accs[b])).then_inc(cp_sem, 1)

            adds(0)
            adds(1)
            copy(0)
            adds(2)
            copy(1)
            # batch 3: two half reductions pipelined with its streaming load
            vector.wait_ge(in3a_sem, 16)
            nc.vector.tensor_add(t3a, xt3a[:, 0:F], xt3a[:, F : 2 * F]).then_inc(
                vv_sem, 1
            )
            vector.wait_ge(in3b_sem, 16)
            nc.vector.tensor_add(t3b, xt3b[:, 0:F], xt3b[:, F : 2 * F]).then_inc(
                vv_sem, 1
            )
            vector.wait_ge(vv_sem, 5)
            nc.vector.tensor_add(totals[3], t3a, t3b).then_inc(vec_sem, 1)
            copy(2)
            copy(3)

        @block.tensor
        def _(tensor):
            for b in range(B_LOC):
                tensor.wait_ge(vec_sem, b + 2)
                nc.tensor.matmul(
                    accs[b], ones, totals[b], start=True, stop=True
                ).then_inc(pe_sem, 1)

    return nc


def _get_nc():
    if "nc" not in _NC_CACHE:
        _NC_CACHE["nc"] = _build_nc()
    return _NC_CACHE["nc"]


def kernel(x, context=None, W=None, b=None, **_unused):
    """Full inputs in, full output out. context/W/b provably do not affect
    the output (softmax over a size-1 axis is identically 1)."""
    x = np.ascontiguousarray(np.asarray(x), dtype=np.float32)
    assert x.shape == (B, T, F), x.shape

    nc = _get_nc()
    in_maps = [{"x": x[i * B_LOC : (i + 1) * B_LOC]} for i in range(N_CORES)]
    res = run_bass_kernel_spmd(nc, in_maps, core_ids=list(range(N_CORES)))
    return np.concatenate([r["out"] for r in res.results], axis=0)


# revision 10
# speedup vs baseline: 1.1156x; 1.0289x over previous
"""Trainium2 Bass kernel for nn_Attention_85813446574600.

Reference computes:
    s_x = x @ W[:F] + b            # [B,T,1]
    s_c = context @ W[F:]          # [C,1]
    scores = s_x + s_c             # [B,T,C,1]
    att = softmax(scores, axis=-1) # softmax over a SIZE-1 axis -> exactly 1.0
    out = einsum('btc,btf->bcf', att, x)

Since softmax over the last (size-1) axis is identically 1.0 for any finite
scores, the output is exactly out[b,c,f] = sum_t x[b,t,f], independent of c
(and of context/W/b entirely).

Device kernel (per core, batch-sharded 32/8 = 4 batches), raw Bass (no Tile
framework -- avoids its entry/exit barrier overhead):

  sync engine   : all input DMAs on the qSP HWDGE ring (two concurrent
                  rings were measured SLOWER: 2x146GB/s vs 1x323GB/s).
                  Partition p holds consecutive T rows, giving
                  per-partition-contiguous 8KB descriptors. The last batch is
                  split into two half-loads so its reduction starts while the
                  second half is still streaming.
  scalar engine : one 512KB output DMA per batch on the qAct ring with
                  4KB-contiguous descriptors.
  vector engine : pre-reduce the T rows in each partition with wide adds,
                  then copy the matmul result PSUM->SBUF duplicated twice
                  side-by-side (stride-0 read) to feed the 4KB-descriptor
                  output DMA.
  tensor engine : ONES[128,128] @ total -> PSUM; an all-ones stationary
                  matrix both sums across partitions and broadcasts the
                  result to all 128 output partitions in one matmul.
"""

import sys

for _p in ("/opt/trn_rl_repo",):
    if _p not in sys.path:
        sys.path.insert(0, _p)

import dataclasses
from contextlib import ExitStack

import numpy as np

import concourse.bass as bass
import concourse.mybir as mybir
from concourse.bass_utils import run_bass_kernel_spmd

# Problem shapes (hardcoded per harness contract)
B, T, C, F = 32, 512, 256, 512
N_CORES = 8
B_LOC = B // N_CORES  # 4 batches per core
P = 128               # SBUF/PSUM partitions
TT = T // P           # 4 T-rows folded into each partition
DT = mybir.dt.float32

_NC_CACHE = {}


def _dup2(ap):
    """View `ap` ([128, N]) as [128, 2, N] reading the same data twice
    (stride-0 middle dim)."""
    a = ap.ap
    return dataclasses.replace(
        ap, ap=type(a)([list(a[0]), [0, 2], list(a[1])])
    )


def _build_nc():
    nc = bass.Bass("TRN2", target_bir_lowering=False)
    x = nc.dram_tensor("x", [B_LOC, T, F], DT, kind="ExternalInput").ap()
    out = nc.dram_tensor("out", [B_LOC, C, F], DT, kind="ExternalOutput").ap()

    with ExitStack() as ctx:
        ec = ctx.enter_context
        ones = ec(nc.sbuf_tensor("ones", [P, P], DT)).ap()
        # b0..b2: one [128, 4*F] tile each; b3: two [128, 2*F] half tiles
        xts = [
            ec(nc.sbuf_tensor(f"xt{b}", [P, TT * F], DT)).ap() for b in range(3)
        ]
        xt3a = ec(nc.sbuf_tensor("xt3a", [P, 2 * F], DT)).ap()
        xt3b = ec(nc.sbuf_tensor("xt3b", [P, 2 * F], DT)).ap()
        pairs = [
            ec(nc.sbuf_tensor(f"pair{b}", [P, 2 * F], DT)).ap() for b in range(3)
        ]
        t3a = ec(nc.sbuf_tensor("t3a", [P, F], DT)).ap()
        t3b = ec(nc.sbuf_tensor("t3b", [P, F], DT)).ap()
        totals = [
            ec(nc.sbuf_tensor(f"total{b}", [P, F], DT)).ap() for b in range(B_LOC)
        ]
        # duplicated result slabs: [128, 2*F] so the out DMA sees 4KB
        # contiguous per partition
        ots = [
            ec(nc.sbuf_tensor(f"ot{b}", [P, 2 * F], DT)).ap() for b in range(B_LOC)
        ]
        accs = [ec(nc.psum_tensor(f"acc{b}", [P, F], DT)).ap() for b in range(B_LOC)]

        in_sems = [ec(nc.semaphore(f"in_sem{b}")) for b in range(3)]
        in3a_sem = ec(nc.semaphore("in3a_sem"))
        in3b_sem = ec(nc.semaphore("in3b_sem"))
        vec_sem = ec(nc.semaphore("vec_sem"))
        vv_sem = ec(nc.semaphore("vv_sem"))
        pe_sem = ec(nc.semaphore("pe_sem"))
        cp_sem = ec(nc.semaphore("cp_sem"))
        osem_sp = ec(nc.semaphore("osem_sp"))
        osem_act = ec(nc.semaphore("osem_act"))

        block = ec(nc.Block())

        def in_dma(eng, b):
            # partition p <- x[b, TT*p : TT*(p+1), :], contiguous 8KB/partition
            src = x[b].rearrange("(p l) f -> p l f", p=P)
            return eng.dma_start(
                xts[b].rearrange("p (l f) -> p l f", l=TT), src
            ).then_inc(in_sems[b], 16)

        def out_dma(eng, b, sem):
            # out[b] rows (2p, 2p+1) <- ot[b] partition p (4KB contiguous)
            dst = out[b].rearrange("(p r) f -> p r f", p=P)
            src = ots[b].rearrange("p (r f) -> p r f", r=2)
            return eng.dma_start(dst, src).then_inc(sem, 16)

        def out_half(eng, b, h, sem):
            # one 128-row half of out[b] (2KB descriptors)
            dst = out[b, h * P : (h + 1) * P, :]
            return eng.dma_start(dst, ots[b][:, 0:F]).then_inc(sem, 16)

        @block.sync
        def _(sync):
            in_dma(sync, 0)
            in_dma(sync, 1)
            in_dma(sync, 2)
            src3 = x[3].rearrange("(h p l) f -> h p l f", h=2, p=P)
            sync.dma_start(
                xt3a.rearrange("p (l f) -> p l f", l=2), src3[0]
            ).then_inc(in3a_sem, 16)
            sync.dma_start(
                xt3b.rearrange("p (l f) -> p l f", l=2), src3[1]
            ).then_inc(in3b_sem, 16)
            # qSP ring is busy with inputs until ~20us; it then picks up
            # out1 and the first half of out3
            sync.wait_ge(cp_sem, 2)
            out_dma(sync, 1, osem_sp)
            sync.wait_ge(cp_sem, 4)
            out_half(sync, 3, 0, osem_sp)
            sync.wait_ge(osem_sp, 32)

        @block.scalar
        def _(scalar):
            # qAct ring is otherwise idle: out0, out2, second half of out3
            scalar.wait_ge(cp_sem, 1)
            out_dma(scalar, 0, osem_act)
            scalar.wait_ge(cp_sem, 3)
            out_dma(scalar, 2, osem_act)
            scalar.wait_ge(cp_sem, 4)
            out_half(scalar, 3, 1, osem_act)
            scalar.wait_ge(osem_act, 48)

        @block.vector
        def _(vector):
            nc.vector.memset(ones, 1.0).then_inc(vec_sem, 1)

            def adds(b):
                vector.wait_ge(in_sems[b], 16)
                nc.vector.tensor_add(
                    pairs[b], xts[b][:, 0 : 2 * F], xts[b][:, 2 * F : 4 * F]
                ).then_inc(vv_sem, 1)
                # same-engine RAW: the DVE pipeline is deep, so the dependent
                # read must wait on the writer's semaphore
                vector.wait_ge(vv_sem, b + 1)
                nc.vector.tensor_add(
                    totals[b], pairs[b][:, 0:F], pairs[b][:, F : 2 * F]
                ).then_inc(vec_sem, 1)

            def copy(b):
                vector.wait_ge(pe_sem, b + 1)
                nc.vector.tensor_copy(ots[b], _dup2(accs[b])).then_inc(cp_sem, 1)

            adds(0)
            adds(1)
            copy(0)
            adds(2)
            # batch 3 adds run BEFORE the remaining copies: totals[3] gates
            # the last matmul, which gates the final output DMA
            vector.wait_ge(in3a_sem, 16)
            nc.vector.tensor_add(t3a, xt3a[:, 0:F], xt3a[:, F : 2 * F]).then_inc(
                vv_sem, 1
            )
            vector.wait_ge(in3b_sem, 16)
            nc.vector.tensor_add(t3b, xt3b[:, 0:F], xt3b[:, F : 2 * F]).then_inc(
                vv_sem, 1
            )
            vector.wait_ge(vv_sem, 5)
            nc.vector.tensor_add(totals[3], t3a, t3b).then_inc(vec_sem, 1)
            copy(1)
            copy(2)
            copy(3)

        @block.tensor
        def _(tensor):
            for b in range(B_LOC):
                tensor.wait_ge(vec_sem, b + 2)
                nc.tensor.matmul(
                    accs[b], ones, totals[b], start=True, stop=True
                ).then_inc(pe_sem, 1)

    return nc


def _get_nc():
    if "nc" not in _NC_CACHE:
        _NC_CACHE["nc"] = _build_nc()
    return _NC_CACHE["nc"]


def kernel(x, context=None, W=None, b=None, **_unused):
    """Full inputs in, full output out. context/W/b provably do not affect
    the output (softmax over a size-1 axis is identically 1)."""
    x = np.ascontiguousarray(np.asarray(x), dtype=np.float32)
    assert x.shape == (B, T, F), x.shape

    nc = _get_nc()
    in_maps = [{"x": x[i * B_LOC : (i + 1) * B_LOC]} for i in range(N_CORES)]
    res = run_bass_kernel_spmd(nc, in_maps, core_ids=list(range(N_CORES)))
    return np.concatenate([r["out"] for r in res.results], axis=0)
